# revision 6
# baseline (speedup 1.0000x reference)
"""LiteGearNet GNN message-passing kernel for 8 Trainium2 NeuronCores.

Strategy (matches the sharding hint: partition nodes, replicate weights):
 - Nodes are partitioned across the 8 cores (6250 each). Each core computes
   its nodes' messages, MLP and LayerNorm. The small 128x128 weights are
   replicated.
 - The gather h[src] is a hardware `dma_gather` (SWDGE) from a per-core
   HBM replica of h (fp16). Edges are pre-sorted by destination window
   (128 dst nodes) on the host, so the scatter-add becomes a sequence of
   one-hot matmuls accumulating in PSUM (S^T @ G per 128-edge chunk),
   with the one-hot S matrices precomputed on the host in fp8 and kept
   resident in SBUF.
 - After each layer, cores AllGather their fp16 shard of the new h into a
   shared HBM buffer that serves as the next layer's gather source.
 - Precision: gather/matmul operands fp16/fp8(one-hot exact), PSUM/MLP
   accumulation + LayerNorm in fp32, residual master copy of h in fp32.
"""
import hashlib
from contextlib import ExitStack
from dataclasses import dataclass, field

import numpy as np
import ml_dtypes

import concourse.bass as bass
import concourse.tile as tile
from concourse import bacc, mybir
from concourse import bass_utils

F32 = mybir.dt.float32
F16 = mybir.dt.float16
F8 = mybir.dt.float8e4
I16 = mybir.dt.int16


@dataclass
class Cfg:
    N: int = 50000          # nodes
    D: int = 128            # feature dim
    LAYERS: int = 3
    CORES: int = 8
    LN_EPS: float = 1e-5
    SPLIT: int = 32768      # int16 gather-table split
    GROUP_W: int = 5        # windows per gather group

    @property
    def NPC(self):          # nodes per core
        return self.N // self.CORES

    @property
    def NWIN(self):         # 128-node windows per core
        return (self.NPC + 127) // 128

    @property
    def WSLOTS(self):       # padded node slots per core
        return self.NWIN * 128

    @property
    def PADN(self):         # padded rows of the replicated h table
        return self.CORES * self.WSLOTS

    @property
    def groups(self):
        gs = []
        w = 0
        while w < self.NWIN:
            gs.append(list(range(w, min(w + self.GROUP_W, self.NWIN))))
            w += self.GROUP_W
        return gs


@dataclass
class Prep:
    caps: np.ndarray            # [NWIN, 2] chunks per (window, half)
    chunk_base: np.ndarray      # [NWIN, 2] global chunk index of first chunk
    gathers: list               # per (group, half): (half, chunk_off, nch)
    CHT: int
    idx_all: np.ndarray         # [CORES, 128, 8*CHT] int16
    S_all: np.ndarray           # [CORES, 128, CHT*128] fp8
    deg_inv: np.ndarray         # [CORES, 128, NWIN] fp32


def preprocess(edge_index: np.ndarray, cfg: Cfg) -> Prep:
    src = np.asarray(edge_index[0]).astype(np.int64)
    dst = np.asarray(edge_index[1]).astype(np.int64)
    deg = np.maximum(np.bincount(dst, minlength=cfg.N), 1).astype(np.float32)

    pad_src = (src // cfg.NPC) * cfg.WSLOTS + (src % cfg.NPC)
    core = dst // cfg.NPC
    local = dst - core * cfg.NPC
    w = local >> 7
    j = local & 127
    half = (pad_src >= cfg.SPLIT).astype(np.int64)
    assert cfg.SPLIT <= 32768 and cfg.PADN - cfg.SPLIT <= 32768, \
        "both gather tables must be indexable by int16"

    order = np.lexsort((pad_src, half, w, core))
    s_src = pad_src[order]
    s_j = j[order]
    key = ((core * cfg.NWIN + w) * 2 + half)[order]
    nkeys = cfg.CORES * cfg.NWIN * 2
    counts = np.bincount(key, minlength=nkeys).reshape(cfg.CORES, cfg.NWIN, 2)
    cum = np.concatenate([[0], np.cumsum(np.bincount(key, minlength=nkeys))])

    caps = -(-counts.max(axis=0) // 128)        # [NWIN, 2] ceil
    # global chunk order: for each group: low chunks (w-major), high chunks
    chunk_base = np.zeros((cfg.NWIN, 2), np.int64)
    gathers = []
    nxt = 0
    for g in cfg.groups:
        for h in (0, 1):
            off = nxt
            for w_ in g:
                chunk_base[w_, h] = nxt
                nxt += caps[w_, h]
            gathers.append((h, off, nxt - off))
    CHT = int(nxt)

    idx_all = np.zeros((cfg.CORES, 128, 8 * CHT), np.int16)
    S_all = np.zeros((cfg.CORES, 128, CHT * 128), ml_dtypes.float8_e4m3)
    deg_inv = np.ones((cfg.CORES, 128, cfg.NWIN), np.float32)

    # gather-local offset of each chunk (for idx wrapping)
    g_of_chunk = np.zeros(CHT, np.int64)     # chunk -> its gather's chunk_off
    for (h, off, nch) in gathers:
        g_of_chunk[off:off + nch] = off

    for k in range(cfg.CORES):
        nd = np.arange(cfg.WSLOTS)
        real = nd < cfg.NPC
        deg_inv[k, nd[real] & 127, nd[real] >> 7] = 1.0 / deg[k * cfg.NPC + nd[real]]
        for w_ in range(cfg.NWIN):
            for h in (0, 1):
                cnt = counts[k, w_, h]
                if cnt == 0:
                    continue
                lo = cum[(k * cfg.NWIN + w_) * 2 + h]
                e_src = s_src[lo:lo + cnt] - (cfg.SPLIT if h else 0)
                e_j = s_j[lo:lo + cnt]
                pos = np.arange(cnt)
                c = chunk_base[w_, h] + (pos >> 7)
                e = pos & 127
                S_all[k, e, c * 128 + e_j] = 1.0
                # idx wrapped within the covering gather instruction
                i_g = (c - g_of_chunk[c]) * 128 + e
                col = 8 * g_of_chunk[c] + (i_g >> 4)
                row = i_g & 15
                for r in range(8):
                    idx_all[k, 16 * r + row, col] = e_src.astype(np.int16)

    return Prep(caps=caps, chunk_base=chunk_base, gathers=gathers, CHT=CHT,
                idx_all=idx_all, S_all=S_all, deg_inv=deg_inv)


def build_program(cfg: Cfg, prep: Prep, apply_gb: bool):
    nc = bacc.Bacc("TRN2", target_bir_lowering=False, debug=False,
                   num_devices=cfg.CORES)
    L, D, CHT, NWIN = cfg.LAYERS, cfg.D, prep.CHT, cfg.NWIN

    # ---- I/O ----
    xT_in = nc.dram_tensor("xT", [128, cfg.WSLOTS], F32, kind="ExternalInput").ap()
    S_in = nc.dram_tensor("S", [128, CHT * 128], F8, kind="ExternalInput").ap()
    idx_in = nc.dram_tensor("idx", [128, 8 * CHT], I16, kind="ExternalInput").ap()
    deg_in = nc.dram_tensor("deg", [128, NWIN], F32, kind="ExternalInput").ap()
    inw_in = nc.dram_tensor("inw", [128, 128], F32, kind="ExternalInput").ap()
    inb_in = nc.dram_tensor("inb", [128, 128], F32, kind="ExternalInput").ap()
    w1_in = nc.dram_tensor("w1", [128, L * 128], F16, kind="ExternalInput").ap()
    w2_in = nc.dram_tensor("w2", [128, L * 128], F16, kind="ExternalInput").ap()
    b1_in = nc.dram_tensor("b1", [128, L], F32, kind="ExternalInput").ap()
    b2_in = nc.dram_tensor("b2", [128, L], F32, kind="ExternalInput").ap()
    id_in = nc.dram_tensor("ident", [128, 128], F16, kind="ExternalInput").ap()
    g_in = nc.dram_tensor("grep", [128, L * 128], F32, kind="ExternalInput").ap()
    bb_in = nc.dram_tensor("brep", [128, L * 128], F32, kind="ExternalInput").ap()
    out_t = nc.dram_tensor("h_out", [cfg.WSLOTS, 128], F32, kind="ExternalOutput").ap()

    # ---- internal DRAM ----
    shards = [nc.dram_tensor(f"shard{l}", [cfg.WSLOTS, 128], F16).ap()
              for l in range(L)]
    hfulls = [nc.dram_tensor(f"hfull{l}", [cfg.PADN, 128], F16,
                             addr_space="Shared").ap()
              for l in range(L)]

    grp_nch = []
    for gi, g in enumerate(cfg.groups):
        nlo = int(prep.caps[g, 0].sum())
        nhi = int(prep.caps[g, 1].sum())
        grp_nch.append((nlo, nhi))
    max_nch = max(a + b for a, b in grp_nch)

    with TileKernel(nc) as tc, ExitStack() as ctx:
        cp = ctx.enter_context(tc.tile_pool(name="const", bufs=1))
        # resident tiles
        S_t = cp.tile([128, CHT * 128], F8)
        idx_t = cp.tile([128, 8 * CHT], I16)
        h_loc = cp.tile([128, NWIN * 128], F32)
        deg_t = cp.tile([128, NWIN], F32)
        inw_t = cp.tile([128, 128], F32)
        inb_t = cp.tile([128, 128], F32)
        w1_t = cp.tile([128, L * 128], F16)
        w2_t = cp.tile([128, L * 128], F16)
        b1_t = cp.tile([128, L], F32)
        b2_t = cp.tile([128, L], F32)
        id_t = cp.tile([128, 128], F16)
        g_t = cp.tile([128, L * 128], F32)
        bb_t = cp.tile([128, L * 128], F32)
        eps_t = cp.tile([128, 1], F32)
        nc.vector.memset(eps_t[:], cfg.LN_EPS)

        nc.sync.dma_start(S_t[:], S_in)
        nc.sync.dma_start(idx_t[:], idx_in)
        nc.sync.dma_start(deg_t[:], deg_in)
        nc.sync.dma_start(inw_t[:], inw_in)
        nc.sync.dma_start(inb_t[:], inb_in)
        nc.sync.dma_start(w1_t[:], w1_in)
        nc.sync.dma_start(w2_t[:], w2_in)
        nc.sync.dma_start(b1_t[:], b1_in)
        nc.sync.dma_start(b2_t[:], b2_in)
        nc.sync.dma_start(id_t[:], id_in)
        if apply_gb:
            nc.sync.dma_start(g_t[:], g_in)
            nc.sync.dma_start(bb_t[:], bb_in)

        # pools
        gp = ctx.enter_context(tc.tile_pool(name="G", bufs=2))
        wp = ctx.enter_context(tc.tile_pool(name="wrk", bufs=3))
        sp = ctx.enter_context(tc.tile_pool(name="small", bufs=4))
        pm = ctx.enter_context(tc.tile_pool(name="pm", bufs=2, space="PSUM"))
        pt = ctx.enter_context(tc.tile_pool(name="pt", bufs=2, space="PSUM"))
        pz = ctx.enter_context(tc.tile_pool(name="pz", bufs=2, space="PSUM"))

        # ---- prologue: h0 = x @ in_w + in_b (node-major windows) ----
        with tc.tile_pool(name="xp", bufs=1) as xp:
            xT_t = xp.tile([128, cfg.WSLOTS], F32)
            nc.sync.dma_start(xT_t[:], xT_in)
            for w_ in range(NWIN):
                h0 = pz.tile([128, 128], F32, tag="mm")
                nc.tensor.matmul(h0[:], xT_t[:, w_ * 128:(w_ + 1) * 128],
                                 inw_t[:], start=True, stop=True)
                nc.vector.tensor_tensor(h_loc[:, w_ * 128:(w_ + 1) * 128],
                                        h0[:], inb_t[:], mybir.AluOpType.add)

        def shard_allgather(l):
            # fp32 SBUF (node window-major) -> fp16 node-major DRAM, then gather
            nc.gpsimd.dma_start(
                shards[l].rearrange("(w p) f -> p w f", p=128),
                h_loc[:].rearrange("p (w f) -> p w f", f=128),
            )
            nc.gpsimd.collective_compute(
                "AllGather", mybir.AluOpType.bypass,
                replica_groups=[list(range(cfg.CORES))],
                ins=[shards[l]], outs=[hfulls[l]],
            )

        shard_allgather(0)

        # ---- layers ----
        for l in range(L):
            hsrc = hfulls[l]
            lo_tbl = hsrc[0:cfg.SPLIT, :]
            hi_tbl = hsrc[cfg.SPLIT:cfg.PADN, :]
            for gi, g in enumerate(cfg.groups):
                nlo, nhi = grp_nch[gi]
                nch = nlo + nhi
                G_t = gp.tile([128, max_nch, D], F16, tag="G")
                goff = int(prep.chunk_base[g[0], 0])  # first chunk of group
                for (h, coff, n) in [(0, goff, nlo), (1, goff + nlo, nhi)]:
                    if n == 0:
                        continue
                    tbl = lo_tbl if h == 0 else hi_tbl
                    # <=8 chunks (1024 idxs = 64 descs/engine) per gather so
                    # single-packet mode stays within the 64-desc packet limit
                    for c0 in range(coff, coff + n, 8):
                        nn_ = min(8, coff + n - c0)
                        nc.gpsimd.dma_gather(
                            G_t[:, c0 - goff:c0 - goff + nn_, :], tbl,
                            idx_t[:, 8 * c0:8 * (c0 + nn_)],
                            num_idxs=128 * nn_, num_idxs_reg=128 * nn_,
                            elem_size=D, single_packet=True,
                        )
                for w_ in g:
                    # scatter: m[dst, f] = sum_c S_c^T @ G_c  (PSUM fp32)
                    chunks = []
                    for h in (0, 1):
                        b0 = int(prep.chunk_base[w_, h])
                        chunks += list(range(b0, b0 + int(prep.caps[w_, h])))
                    m_ps = pm.tile([128, D], F32, tag="m")
                    for ci, c in enumerate(chunks):
                        nc.tensor.matmul(
                            m_ps[:], S_t[:, c * 128:(c + 1) * 128],
                            G_t[:, c - goff, :],
                            start=(ci == 0), stop=(ci == len(chunks) - 1),
                        )
                    # mhat = m * deg_inv  (per-dst scalar), evac -> fp16
                    mhat = wp.tile([128, D], F16, tag="mhat")
                    nc.vector.tensor_scalar(mhat[:], m_ps[:],
                                            deg_t[:, w_:w_ + 1], None,
                                            mybir.AluOpType.mult)
                    # transpose to [feat, nodes]
                    mT_ps = pt.tile([128, D], F16, tag="tr")
                    nc.tensor.transpose(mT_ps[:], mhat[:], id_t[:])
                    mT = wp.tile([128, D], F16, tag="mTs")
                    nc.scalar.copy(mT[:], mT_ps[:])
                    # z1 = relu(W1^T mT + b1)
                    z1_ps = pz.tile([128, D], F32, tag="mm")
                    nc.tensor.matmul(z1_ps[:], w1_t[:, l * 128:(l + 1) * 128],
                                     mT[:], start=True, stop=True)
                    z1 = wp.tile([128, D], F16, tag="z1s")
                    nc.scalar.activation(z1[:], z1_ps[:],
                                         mybir.ActivationFunctionType.Relu,
                                         bias=b1_t[:, l:l + 1])
                    # z2 = W2^T z1 + b2
                    z2_ps = pz.tile([128, D], F32, tag="mm")
                    nc.tensor.matmul(z2_ps[:], w2_t[:, l * 128:(l + 1) * 128],
                                     z1[:], start=True, stop=True)
                    z2t = wp.tile([128, D], F16, tag="z2t")
                    nc.scalar.activation(z2t[:], z2_ps[:],
                                         mybir.ActivationFunctionType.Identity,
                                         bias=b2_t[:, l:l + 1])
                    # back to [nodes, feat]
                    z2n_ps = pt.tile([128, D], F16, tag="tr")
                    nc.tensor.transpose(z2n_ps[:], z2t[:], id_t[:])
                    # residual + stats
                    hw = h_loc[:, w_ * 128:(w_ + 1) * 128]
                    r = wp.tile([128, D], F32, tag="r")
                    rs = sp.tile([128, 1], F32, tag="rs")
                    nc.vector.scalar_tensor_tensor(
                        r[:], hw, 0.0, z2n_ps[:],
                        mybir.AluOpType.bypass, mybir.AluOpType.add,
                        accum_out=rs[:])
                    rsq = wp.tile([128, D], F32, tag="rsq")
                    rqs = sp.tile([128, 1], F32, tag="rqs")
                    nc.scalar.activation(rsq[:], r[:],
                                         mybir.ActivationFunctionType.Square,
                                         accum_out=rqs[:])
                    mu = sp.tile([128, 1], F32, tag="mu")
                    nc.vector.tensor_scalar(mu[:], rs[:], 1.0 / D, None,
                                            mybir.AluOpType.mult)
                    mu2 = sp.tile([128, 1], F32, tag="mu2")
                    nc.vector.tensor_tensor(mu2[:], mu[:], mu[:],
                                            mybir.AluOpType.mult)
                    var = sp.tile([128, 1], F32, tag="var")
                    nc.vector.scalar_tensor_tensor(
                        var[:], rqs[:], 1.0 / D, mu2[:],
                        mybir.AluOpType.mult, mybir.AluOpType.subtract)
                    sd = sp.tile([128, 1], F32, tag="sd")
                    nc.scalar.activation(sd[:], var[:],
                                         mybir.ActivationFunctionType.Sqrt,
                                         bias=eps_t[:])
                    inv = sp.tile([128, 1], F32, tag="inv")
                    nc.vector.reciprocal(inv[:], sd[:])
                    if apply_gb:
                        t1 = wp.tile([128, D], F32, tag="t1")
                        nc.vector.tensor_scalar(t1[:], r[:], mu[:], inv[:],
                                                mybir.AluOpType.subtract,
                                                mybir.AluOpType.mult)
                        t2 = wp.tile([128, D], F32, tag="t2")
                        nc.vector.tensor_tensor(
                            t2[:], t1[:], g_t[:, l * 128:(l + 1) * 128],
                            mybir.AluOpType.mult)
                        nc.vector.tensor_tensor(
                            hw, t2[:], bb_t[:, l * 128:(l + 1) * 128],
                            mybir.AluOpType.add)
                    else:
                        nc.vector.tensor_scalar(hw, r[:], mu[:], inv[:],
                                                mybir.AluOpType.subtract,
                                                mybir.AluOpType.mult)
            if l + 1 < L:
                shard_allgather(l + 1)

        # ---- epilogue: write fp32 shard ----
        nc.sync.dma_start(
            out_t.rearrange("(w p) f -> p w f", p=128),
            h_loc[:].rearrange("p (w f) -> p w f", f=128),
        )

    nc.compile()
    return nc


def TileKernel(nc):
    return tile.TileContext(nc)


_CACHE = {}


def _prepare(inputs, cfg: Cfg):
    edge_index = np.asarray(inputs["edge_index"])
    key = hashlib.sha1(edge_index.tobytes()).hexdigest()
    ln_g = np.asarray(inputs["ln_g"], np.float32)
    ln_b = np.asarray(inputs["ln_b"], np.float32)
    apply_gb = not (np.all(ln_g == 1.0) and np.all(ln_b == 0.0))
    key = (key, apply_gb, cfg.N, cfg.CORES)
    if key not in _CACHE:
        prep = preprocess(edge_index, cfg)
        nc = build_program(cfg, prep, apply_gb)
        _CACHE[key] = (prep, nc, apply_gb)
    return _CACHE[key]


def _in_maps(inputs, cfg: Cfg, prep: Prep, apply_gb: bool):
    x = np.asarray(inputs["x"], np.float32)
    L = cfg.LAYERS
    inw = np.asarray(inputs["in_w"], np.float32)
    inb = np.asarray(inputs["in_b"], np.float32)
    w1 = np.asarray(inputs["w1"], np.float32)
    w2 = np.asarray(inputs["w2"], np.float32)
    b1 = np.asarray(inputs["b1"], np.float32)
    b2 = np.asarray(inputs["b2"], np.float32)
    ln_g = np.asarray(inputs["ln_g"], np.float32)
    ln_b = np.asarray(inputs["ln_b"], np.float32)

    inb_rep = np.tile(inb[None, :], (128, 1))
    w1_pack = np.concatenate([w1[l] for l in range(L)], axis=1).astype(np.float16)
    w2_pack = np.concatenate([w2[l] for l in range(L)], axis=1).astype(np.float16)
    b1_cols = np.ascontiguousarray(b1.T)           # [128, L]
    b2_cols = np.ascontiguousarray(b2.T)
    g_rep = np.concatenate([np.tile(ln_g[l][None, :], (128, 1)) for l in range(L)], axis=1)
    b_rep = np.concatenate([np.tile(ln_b[l][None, :], (128, 1)) for l in range(L)], axis=1)
    ident = np.eye(128, dtype=np.float16)

    maps = []
    for k in range(cfg.CORES):
        xT = np.zeros((128, cfg.WSLOTS), np.float32)
        xT[:, :cfg.NPC] = x[k * cfg.NPC:(k + 1) * cfg.NPC].T
        maps.append({
            "xT": xT,
            "S": np.ascontiguousarray(prep.S_all[k]),
            "idx": np.ascontiguousarray(prep.idx_all[k]),
            "deg": np.ascontiguousarray(prep.deg_inv[k]),
            "inw": inw, "inb": inb_rep,
            "w1": w1_pack, "w2": w2_pack,
            "b1": b1_cols, "b2": b2_cols,
            "ident": ident, "grep": g_rep, "brep": b_rep,
        })
    return maps


def _run(inputs, cfg=None, trace=False):
    cfg = cfg or Cfg()
    prep, nc, apply_gb = _prepare(inputs, cfg)
    maps = _in_maps(inputs, cfg, prep, apply_gb)
    res = bass_utils.run_bass_kernel_spmd(
        nc, maps, core_ids=list(range(cfg.CORES)),
        trace=trace, trace_cores=list(range(cfg.CORES)) if trace else None,
    )
    out = np.concatenate(
        [res.results[k]["h_out"][:cfg.NPC] for k in range(cfg.CORES)], axis=0)
    return out.astype(np.float32), res


def kernel(**inputs) -> np.ndarray:
    out, _ = _run(inputs)
    return out


def kernel_profiled(**inputs):
    out, res = _run(inputs, trace=True)
    return out, res


# revision 8
# speedup vs baseline: 1.0072x; 1.0072x over previous
"""LiteGearNet GNN message-passing kernel for 8 Trainium2 NeuronCores.

Strategy (matches the sharding hint: partition nodes, replicate weights):
 - Nodes are partitioned across the 8 cores (6250 each). Each core computes
   its nodes' messages, MLP and LayerNorm. The small 128x128 weights are
   replicated.
 - The gather h[src] is a hardware `dma_gather` (SWDGE) from a per-core
   HBM replica of h (fp16). Edges are pre-sorted by destination window
   (128 dst nodes) on the host, so the scatter-add becomes a sequence of
   one-hot matmuls accumulating in PSUM (S^T @ G per 128-edge chunk),
   with the one-hot S matrices precomputed on the host in fp8 and kept
   resident in SBUF.
 - After each layer, cores AllGather their fp16 shard of the new h into a
   shared HBM buffer that serves as the next layer's gather source.
 - Precision: gather/matmul operands fp16/fp8(one-hot exact), PSUM/MLP
   accumulation + LayerNorm in fp32, residual master copy of h in fp32.
"""
import hashlib
from contextlib import ExitStack
from dataclasses import dataclass, field

import numpy as np
import ml_dtypes

import concourse.bass as bass
import concourse.tile as tile
from concourse import bacc, mybir
from concourse import bass_utils

F32 = mybir.dt.float32
F16 = mybir.dt.float16
F8 = mybir.dt.float8e4
I16 = mybir.dt.int16


@dataclass
class Cfg:
    N: int = 50000          # nodes
    D: int = 128            # feature dim
    LAYERS: int = 3
    CORES: int = 8
    LN_EPS: float = 1e-5
    SPLIT: int = 32768      # int16 gather-table split
    GROUP_W: int = 3        # windows per gather group

    @property
    def NPC(self):          # nodes per core
        return self.N // self.CORES

    @property
    def NWIN(self):         # 128-node windows per core
        return (self.NPC + 127) // 128

    @property
    def WSLOTS(self):       # padded node slots per core
        return self.NWIN * 128

    @property
    def PADN(self):         # padded rows of the replicated h table
        return self.CORES * self.WSLOTS

    @property
    def groups(self):
        gs = []
        w = 0
        while w < self.NWIN:
            gs.append(list(range(w, min(w + self.GROUP_W, self.NWIN))))
            w += self.GROUP_W
        return gs


@dataclass
class Prep:
    caps: np.ndarray            # [NWIN, 2] chunks per (window, half)
    chunk_base: np.ndarray      # [NWIN, 2] global chunk index of first chunk
    gathers: list               # per (group, half): (half, chunk_off, nch)
    CHT: int
    idx_all: np.ndarray         # [CORES, 128, 8*CHT] int16
    S_all: np.ndarray           # [CORES, 128, CHT*128] fp8
    deg_inv: np.ndarray         # [CORES, 128, NWIN] fp32
    slot_node: np.ndarray       # [CORES, WSLOTS] node id per slot (-1 pad)


def _balanced_slots(deg: np.ndarray, cfg: Cfg) -> np.ndarray:
    """Assign nodes to (core, window, slot) balancing per-window in-degree
    load. Returns slot_node [CORES, WSLOTS] with -1 padding."""
    import heapq
    nwin_all = cfg.CORES * cfg.NWIN
    loads = [(0, wg) for wg in range(nwin_all)]
    heapq.heapify(loads)
    slots_left = np.full(nwin_all, 128, np.int64)
    core_left = np.full(cfg.CORES, cfg.NPC, np.int64)
    slot_node = np.full((cfg.CORES, cfg.WSLOTS), -1, np.int64)
    order = np.argsort(-deg, kind="stable")
    for n in order:
        held = []
        while True:
            load, wg = heapq.heappop(loads)
            k = wg // cfg.NWIN
            if slots_left[wg] > 0 and core_left[k] > 0:
                w_ = wg % cfg.NWIN
                p = 128 - slots_left[wg]
                slot_node[k, w_ * 128 + p] = n
                slots_left[wg] -= 1
                core_left[k] -= 1
                heapq.heappush(loads, (load + int(deg[n]), wg))
                break
            held.append((load, wg))
            # window/core full: drop it from the heap permanently if full
            if slots_left[wg] > 0 and core_left[k] == 0:
                continue
        for it in held:
            k2 = it[1] // cfg.NWIN
            if slots_left[it[1]] > 0 and core_left[k2] > 0:
                heapq.heappush(loads, it)
    return slot_node


def preprocess(edge_index: np.ndarray, cfg: Cfg) -> Prep:
    src = np.asarray(edge_index[0]).astype(np.int64)
    dst = np.asarray(edge_index[1]).astype(np.int64)
    degc = np.bincount(dst, minlength=cfg.N)
    deg = np.maximum(degc, 1).astype(np.float32)
    assert cfg.SPLIT <= 32768 and cfg.PADN - cfg.SPLIT <= 32768, \
        "both gather tables must be indexable by int16"

    slot_node = _balanced_slots(degc.astype(np.int64), cfg)
    # node -> global slot id
    gslot = np.zeros(cfg.N, np.int64)
    flat = slot_node.reshape(-1)
    valid = flat >= 0
    gslot[flat[valid]] = np.arange(cfg.CORES * cfg.WSLOTS)[valid]

    pad_src = gslot[src]
    dslot = gslot[dst]
    core = dslot // cfg.WSLOTS
    local = dslot - core * cfg.WSLOTS
    w = local >> 7
    j = local & 127
    half = (pad_src >= cfg.SPLIT).astype(np.int64)

    # sort by (core, w, half, src) then dedup identical (src) per (core,w,half)
    order = np.lexsort((j, pad_src, half, w, core))
    s_src = pad_src[order]
    s_j = j[order]
    s_key = ((core * cfg.NWIN + w) * 2 + half)[order]
    # unique (key, src) pairs -> gather items; each item scatters to >=1 dst
    new_item = np.concatenate(
        [[True], (s_key[1:] != s_key[:-1]) | (s_src[1:] != s_src[:-1])])
    item_id = np.cumsum(new_item) - 1          # per edge -> item
    n_items = int(item_id[-1]) + 1
    it_src = s_src[new_item]
    it_key = s_key[new_item]
    nkeys = cfg.CORES * cfg.NWIN * 2
    counts = np.bincount(it_key, minlength=nkeys).reshape(cfg.CORES, cfg.NWIN, 2)
    cum = np.concatenate([[0], np.cumsum(np.bincount(it_key, minlength=nkeys))])

    caps = -(-counts.max(axis=0) // 128)        # [NWIN, 2] ceil over cores
    chunk_base = np.zeros((cfg.NWIN, 2), np.int64)
    gathers = []
    nxt = 0
    for g in cfg.groups:
        for h in (0, 1):
            off = nxt
            for w_ in g:
                chunk_base[w_, h] = nxt
                nxt += caps[w_, h]
            gathers.append((h, off, nxt - off))
    CHT = int(nxt)

    idx_all = np.zeros((cfg.CORES, 128, 8 * CHT), np.int16)
    S_host = np.zeros((cfg.CORES, 128, CHT * 128), np.float16)
    deg_inv = np.ones((cfg.CORES, 128, cfg.NWIN), np.float32)

    g_of_chunk = np.zeros(CHT, np.int64)
    for (h, off, nch) in gathers:
        g_of_chunk[off:off + nch] = off

    # per-item slot within its (k,w,h) run
    item_pos = np.arange(n_items) - cum[it_key]
    # edge -> its item's (chunk, part)
    kk = it_key // (cfg.NWIN * 2)
    ww = (it_key // 2) % cfg.NWIN
    hh = it_key & 1
    it_chunk = chunk_base[ww, hh] + (item_pos >> 7)
    it_part = item_pos & 127
    # scatter S multiplicities (edge-level, duplicates accumulate)
    e_chunk = it_chunk[item_id]
    e_part = it_part[item_id]
    e_core = kk[item_id]
    np.add.at(S_host, (e_core, e_part, e_chunk * 128 + s_j), 1.0)
    # gather idx tables (item-level)
    i_g = (it_chunk - g_of_chunk[it_chunk]) * 128 + it_part
    col = 8 * g_of_chunk[it_chunk] + (i_g >> 4)
    row = i_g & 15
    idx_val = (it_src - np.where(hh == 1, cfg.SPLIT, 0)).astype(np.int16)
    for r in range(8):
        idx_all[kk, 16 * r + row, col] = idx_val

    for k in range(cfg.CORES):
        nodes = slot_node[k]
        real = nodes >= 0
        nd = np.arange(cfg.WSLOTS)[real]
        deg_inv[k, nd & 127, nd >> 7] = 1.0 / deg[nodes[real]]

    return Prep(caps=caps, chunk_base=chunk_base, gathers=gathers, CHT=CHT,
                idx_all=idx_all, S_all=S_host.astype(ml_dtypes.float8_e4m3),
                deg_inv=deg_inv, slot_node=slot_node)


def build_program(cfg: Cfg, prep: Prep, apply_gb: bool):
    nc = bacc.Bacc("TRN2", target_bir_lowering=False, debug=False,
                   num_devices=cfg.CORES)
    L, D, CHT, NWIN = cfg.LAYERS, cfg.D, prep.CHT, cfg.NWIN

    # ---- I/O ----
    xT_in = nc.dram_tensor("xT", [128, cfg.WSLOTS], F32, kind="ExternalInput").ap()
    S_in = nc.dram_tensor("S", [128, CHT * 128], F8, kind="ExternalInput").ap()
    idx_in = nc.dram_tensor("idx", [128, 8 * CHT], I16, kind="ExternalInput").ap()
    deg_in = nc.dram_tensor("deg", [128, NWIN], F32, kind="ExternalInput").ap()
    inw_in = nc.dram_tensor("inw", [128, 128], F32, kind="ExternalInput").ap()
    inb_in = nc.dram_tensor("inb", [128, 128], F32, kind="ExternalInput").ap()
    w1_in = nc.dram_tensor("w1", [128, L * 128], F16, kind="ExternalInput").ap()
    w2_in = nc.dram_tensor("w2", [128, L * 128], F16, kind="ExternalInput").ap()
    b1_in = nc.dram_tensor("b1", [128, L], F32, kind="ExternalInput").ap()
    b2_in = nc.dram_tensor("b2", [128, L], F32, kind="ExternalInput").ap()
    id_in = nc.dram_tensor("ident", [128, 128], F16, kind="ExternalInput").ap()
    g_in = nc.dram_tensor("grep", [128, L * 128], F32, kind="ExternalInput").ap()
    bb_in = nc.dram_tensor("brep", [128, L * 128], F32, kind="ExternalInput").ap()
    out_t = nc.dram_tensor("h_out", [cfg.WSLOTS, 128], F32, kind="ExternalOutput").ap()

    # ---- internal DRAM ----
    shards = [nc.dram_tensor(f"shard{l}", [cfg.WSLOTS, 128], F16).ap()
              for l in range(L)]
    hfulls = [nc.dram_tensor(f"hfull{l}", [cfg.PADN, 128], F16,
                             addr_space="Shared").ap()
              for l in range(L)]

    grp_nch = []
    for gi, g in enumerate(cfg.groups):
        nlo = int(prep.caps[g, 0].sum())
        nhi = int(prep.caps[g, 1].sum())
        grp_nch.append((nlo, nhi))
    max_nch = max(a + b for a, b in grp_nch)

    with TileKernel(nc) as tc, ExitStack() as ctx:
        cp = ctx.enter_context(tc.tile_pool(name="const", bufs=1))
        # resident tiles
        S_t = cp.tile([128, CHT * 128], F8)
        idx_t = cp.tile([128, 8 * CHT], I16)
        h_loc = cp.tile([128, NWIN * 128], F32)
        deg_t = cp.tile([128, NWIN], F32)
        inw_t = cp.tile([128, 128], F32)
        inb_t = cp.tile([128, 128], F32)
        w1_t = cp.tile([128, L * 128], F16)
        w2_t = cp.tile([128, L * 128], F16)
        b1_t = cp.tile([128, L], F32)
        b2_t = cp.tile([128, L], F32)
        id_t = cp.tile([128, 128], F16)
        g_t = cp.tile([128, L * 128], F32)
        bb_t = cp.tile([128, L * 128], F32)
        eps_t = cp.tile([128, 1], F32)
        nc.vector.memset(eps_t[:], cfg.LN_EPS)

        nc.sync.dma_start(S_t[:], S_in)
        nc.sync.dma_start(idx_t[:], idx_in)
        nc.sync.dma_start(deg_t[:], deg_in)
        nc.sync.dma_start(inw_t[:], inw_in)
        nc.sync.dma_start(inb_t[:], inb_in)
        nc.sync.dma_start(w1_t[:], w1_in)
        nc.sync.dma_start(w2_t[:], w2_in)
        nc.sync.dma_start(b1_t[:], b1_in)
        nc.sync.dma_start(b2_t[:], b2_in)
        nc.sync.dma_start(id_t[:], id_in)
        if apply_gb:
            nc.sync.dma_start(g_t[:], g_in)
            nc.sync.dma_start(bb_t[:], bb_in)

        # pools
        gp = ctx.enter_context(tc.tile_pool(name="G", bufs=3))
        wp = ctx.enter_context(tc.tile_pool(name="wrk", bufs=3))
        sp = ctx.enter_context(tc.tile_pool(name="small", bufs=4))
        pm = ctx.enter_context(tc.tile_pool(name="pm", bufs=2, space="PSUM"))
        pt = ctx.enter_context(tc.tile_pool(name="pt", bufs=2, space="PSUM"))
        pz = ctx.enter_context(tc.tile_pool(name="pz", bufs=2, space="PSUM"))

        # ---- prologue: h0 = x @ in_w + in_b (node-major windows) ----
        with tc.tile_pool(name="xp", bufs=1) as xp:
            xT_t = xp.tile([128, cfg.WSLOTS], F32)
            nc.sync.dma_start(xT_t[:], xT_in)
            for w_ in range(NWIN):
                h0 = pz.tile([128, 128], F32, tag="mm")
                nc.tensor.matmul(h0[:], xT_t[:, w_ * 128:(w_ + 1) * 128],
                                 inw_t[:], start=True, stop=True)
                nc.vector.tensor_tensor(h_loc[:, w_ * 128:(w_ + 1) * 128],
                                        h0[:], inb_t[:], mybir.AluOpType.add)

        def shard_allgather(l):
            # fp32 SBUF (node window-major) -> fp16 node-major DRAM, then gather
            nc.gpsimd.dma_start(
                shards[l].rearrange("(w p) f -> p w f", p=128),
                h_loc[:].rearrange("p (w f) -> p w f", f=128),
            )
            nc.gpsimd.collective_compute(
                "AllGather", mybir.AluOpType.bypass,
                replica_groups=[list(range(cfg.CORES))],
                ins=[shards[l]], outs=[hfulls[l]],
            )

        shard_allgather(0)

        # ---- layers ----
        for l in range(L):
            hsrc = hfulls[l]
            lo_tbl = hsrc[0:cfg.SPLIT, :]
            hi_tbl = hsrc[cfg.SPLIT:cfg.PADN, :]
            for gi, g in enumerate(cfg.groups):
                nlo, nhi = grp_nch[gi]
                nch = nlo + nhi
                G_t = gp.tile([128, max_nch, D], F16, tag="G")
                goff = int(prep.chunk_base[g[0], 0])  # first chunk of group
                for (h, coff, n) in [(0, goff, nlo), (1, goff + nlo, nhi)]:
                    if n == 0:
                        continue
                    tbl = lo_tbl if h == 0 else hi_tbl
                    # <=8 chunks (1024 idxs = 64 descs/engine) per gather so
                    # single-packet mode stays within the 64-desc packet limit
                    for c0 in range(coff, coff + n, 8):
                        nn_ = min(8, coff + n - c0)
                        nc.gpsimd.dma_gather(
                            G_t[:, c0 - goff:c0 - goff + nn_, :], tbl,
                            idx_t[:, 8 * c0:8 * (c0 + nn_)],
                            num_idxs=128 * nn_, num_idxs_reg=128 * nn_,
                            elem_size=D, single_packet=True,
                        )
                for w_ in g:
                    # scatter: m[dst, f] = sum_c S_c^T @ G_c  (PSUM fp32)
                    chunks = []
                    for h in (0, 1):
                        b0 = int(prep.chunk_base[w_, h])
                        chunks += list(range(b0, b0 + int(prep.caps[w_, h])))
                    m_ps = pm.tile([128, D], F32, tag="m")
                    for ci, c in enumerate(chunks):
                        nc.tensor.matmul(
                            m_ps[:], S_t[:, c * 128:(c + 1) * 128],
                            G_t[:, c - goff, :],
                            start=(ci == 0), stop=(ci == len(chunks) - 1),
                        )
                    # mhat = m * deg_inv  (per-dst scalar), evac -> fp16
                    mhat = wp.tile([128, D], F16, tag="mhat")
                    nc.vector.tensor_scalar(mhat[:], m_ps[:],
                                            deg_t[:, w_:w_ + 1], None,
                                            mybir.AluOpType.mult)
                    # transpose to [feat, nodes]
                    mT_ps = pt.tile([128, D], F16, tag="tr")
                    nc.tensor.transpose(mT_ps[:], mhat[:], id_t[:])
                    mT = wp.tile([128, D], F16, tag="mTs")
                    nc.scalar.copy(mT[:], mT_ps[:])
                    # z1 = relu(W1^T mT + b1)
                    z1_ps = pz.tile([128, D], F32, tag="mm")
                    nc.tensor.matmul(z1_ps[:], w1_t[:, l * 128:(l + 1) * 128],
                                     mT[:], start=True, stop=True)
                    z1 = wp.tile([128, D], F16, tag="z1s")
                    nc.scalar.activation(z1[:], z1_ps[:],
                                         mybir.ActivationFunctionType.Relu,
                                         bias=b1_t[:, l:l + 1])
                    # z2 = W2^T z1 + b2
                    z2_ps = pz.tile([128, D], F32, tag="mm")
                    nc.tensor.matmul(z2_ps[:], w2_t[:, l * 128:(l + 1) * 128],
                                     z1[:], start=True, stop=True)
                    z2t = wp.tile([128, D], F16, tag="z2t")
                    nc.scalar.activation(z2t[:], z2_ps[:],
                                         mybir.ActivationFunctionType.Identity,
                                         bias=b2_t[:, l:l + 1])
                    # back to [nodes, feat]
                    z2n_ps = pt.tile([128, D], F16, tag="tr")
                    nc.tensor.transpose(z2n_ps[:], z2t[:], id_t[:])
                    # residual + stats
                    hw = h_loc[:, w_ * 128:(w_ + 1) * 128]
                    r = wp.tile([128, D], F32, tag="r")
                    rs = sp.tile([128, 1], F32, tag="rs")
                    nc.vector.scalar_tensor_tensor(
                        r[:], hw, 0.0, z2n_ps[:],
                        mybir.AluOpType.bypass, mybir.AluOpType.add,
                        accum_out=rs[:])
                    rsq = wp.tile([128, D], F32, tag="rsq")
                    rqs = sp.tile([128, 1], F32, tag="rqs")
                    nc.scalar.activation(rsq[:], r[:],
                                         mybir.ActivationFunctionType.Square,
                                         accum_out=rqs[:])
                    mu = sp.tile([128, 1], F32, tag="mu")
                    nc.vector.tensor_scalar(mu[:], rs[:], 1.0 / D, None,
                                            mybir.AluOpType.mult)
                    mu2 = sp.tile([128, 1], F32, tag="mu2")
                    nc.vector.tensor_tensor(mu2[:], mu[:], mu[:],
                                            mybir.AluOpType.mult)
                    var = sp.tile([128, 1], F32, tag="var")
                    nc.vector.scalar_tensor_tensor(
                        var[:], rqs[:], 1.0 / D, mu2[:],
                        mybir.AluOpType.mult, mybir.AluOpType.subtract)
                    sd = sp.tile([128, 1], F32, tag="sd")
                    nc.scalar.activation(sd[:], var[:],
                                         mybir.ActivationFunctionType.Sqrt,
                                         bias=eps_t[:])
                    inv = sp.tile([128, 1], F32, tag="inv")
                    nc.vector.reciprocal(inv[:], sd[:])
                    if apply_gb:
                        t1 = wp.tile([128, D], F32, tag="t1")
                        nc.vector.tensor_scalar(t1[:], r[:], mu[:], inv[:],
                                                mybir.AluOpType.subtract,
                                                mybir.AluOpType.mult)
                        t2 = wp.tile([128, D], F32, tag="t2")
                        nc.vector.tensor_tensor(
                            t2[:], t1[:], g_t[:, l * 128:(l + 1) * 128],
                            mybir.AluOpType.mult)
                        nc.vector.tensor_tensor(
                            hw, t2[:], bb_t[:, l * 128:(l + 1) * 128],
                            mybir.AluOpType.add)
                    else:
                        nc.vector.tensor_scalar(hw, r[:], mu[:], inv[:],
                                                mybir.AluOpType.subtract,
                                                mybir.AluOpType.mult)
            if l + 1 < L:
                shard_allgather(l + 1)

        # ---- epilogue: write fp32 shard ----
        nc.sync.dma_start(
            out_t.rearrange("(w p) f -> p w f", p=128),
            h_loc[:].rearrange("p (w f) -> p w f", f=128),
        )

    nc.compile()
    return nc


def TileKernel(nc):
    return tile.TileContext(nc)


_CACHE = {}


def _prepare(inputs, cfg: Cfg):
    edge_index = np.asarray(inputs["edge_index"])
    key = hashlib.sha1(edge_index.tobytes()).hexdigest()
    ln_g = np.asarray(inputs["ln_g"], np.float32)
    ln_b = np.asarray(inputs["ln_b"], np.float32)
    apply_gb = not (np.all(ln_g == 1.0) and np.all(ln_b == 0.0))
    key = (key, apply_gb, cfg.N, cfg.CORES)
    if key not in _CACHE:
        prep = preprocess(edge_index, cfg)
        nc = build_program(cfg, prep, apply_gb)
        _CACHE[key] = (prep, nc, apply_gb)
    return _CACHE[key]


def _in_maps(inputs, cfg: Cfg, prep: Prep, apply_gb: bool):
    x = np.asarray(inputs["x"], np.float32)
    L = cfg.LAYERS
    inw = np.asarray(inputs["in_w"], np.float32)
    inb = np.asarray(inputs["in_b"], np.float32)
    w1 = np.asarray(inputs["w1"], np.float32)
    w2 = np.asarray(inputs["w2"], np.float32)
    b1 = np.asarray(inputs["b1"], np.float32)
    b2 = np.asarray(inputs["b2"], np.float32)
    ln_g = np.asarray(inputs["ln_g"], np.float32)
    ln_b = np.asarray(inputs["ln_b"], np.float32)

    inb_rep = np.tile(inb[None, :], (128, 1))
    w1_pack = np.concatenate([w1[l] for l in range(L)], axis=1).astype(np.float16)
    w2_pack = np.concatenate([w2[l] for l in range(L)], axis=1).astype(np.float16)
    b1_cols = np.ascontiguousarray(b1.T)           # [128, L]
    b2_cols = np.ascontiguousarray(b2.T)
    g_rep = np.concatenate([np.tile(ln_g[l][None, :], (128, 1)) for l in range(L)], axis=1)
    b_rep = np.concatenate([np.tile(ln_b[l][None, :], (128, 1)) for l in range(L)], axis=1)
    ident = np.eye(128, dtype=np.float16)

    maps = []
    for k in range(cfg.CORES):
        xT = np.zeros((128, cfg.WSLOTS), np.float32)
        nodes = prep.slot_node[k]
        vmask = nodes >= 0
        xT[:, vmask] = x[nodes[vmask]].T
        maps.append({
            "xT": xT,
            "S": np.ascontiguousarray(prep.S_all[k]),
            "idx": np.ascontiguousarray(prep.idx_all[k]),
            "deg": np.ascontiguousarray(prep.deg_inv[k]),
            "inw": inw, "inb": inb_rep,
            "w1": w1_pack, "w2": w2_pack,
            "b1": b1_cols, "b2": b2_cols,
            "ident": ident, "grep": g_rep, "brep": b_rep,
        })
    return maps


def _run(inputs, cfg=None, trace=False):
    cfg = cfg or Cfg()
    prep, nc, apply_gb = _prepare(inputs, cfg)
    maps = _in_maps(inputs, cfg, prep, apply_gb)
    res = bass_utils.run_bass_kernel_spmd(
        nc, maps, core_ids=list(range(cfg.CORES)),
        trace=trace, trace_cores=list(range(cfg.CORES)) if trace else None,
    )
    out = np.empty((cfg.N, cfg.D), np.float32)
    for k in range(cfg.CORES):
        nodes = prep.slot_node[k]
        vmask = nodes >= 0
        out[nodes[vmask]] = res.results[k]["h_out"][vmask]
    return out, res


def kernel(**inputs) -> np.ndarray:
    out, _ = _run(inputs)
    return out


def kernel_profiled(**inputs):
    out, res = _run(inputs, trace=True)
    return out, res


# revision 9
# speedup vs baseline: 1.8616x; 1.8483x over previous
"""LiteGearNet GNN message-passing kernel for 8 Trainium2 NeuronCores.

Strategy (matches the sharding hint: partition nodes, replicate weights):
 - Nodes are partitioned across the 8 cores (6250 each). Each core computes
   its nodes' messages, MLP and LayerNorm. The small 128x128 weights are
   replicated.
 - The gather h[src] is a hardware `dma_gather` (SWDGE) from a per-core
   HBM replica of h (fp16). Edges are pre-sorted by destination window
   (128 dst nodes) on the host, so the scatter-add becomes a sequence of
   one-hot matmuls accumulating in PSUM (S^T @ G per 128-edge chunk),
   with the one-hot S matrices precomputed on the host in fp8 and kept
   resident in SBUF.
 - After each layer, cores AllGather their fp16 shard of the new h into a
   shared HBM buffer that serves as the next layer's gather source.
 - Precision: gather/matmul operands fp16/fp8(one-hot exact), PSUM/MLP
   accumulation + LayerNorm in fp32, residual master copy of h in fp32.
"""
import hashlib
from contextlib import ExitStack
from dataclasses import dataclass, field

import numpy as np
import ml_dtypes

import concourse.bass as bass
import concourse.tile as tile
from concourse import bacc, mybir
from concourse import bass_utils

F32 = mybir.dt.float32
F16 = mybir.dt.float16
F8 = mybir.dt.float8e4
I16 = mybir.dt.int16


@dataclass
class Cfg:
    N: int = 50000          # nodes
    D: int = 128            # feature dim
    LAYERS: int = 3
    CORES: int = 8
    LN_EPS: float = 1e-5
    SPLIT: int = 32768      # int16 gather-table split
    GROUP_W: int = 3        # windows per gather group

    @property
    def NPC(self):          # nodes per core
        return self.N // self.CORES

    @property
    def NWIN(self):         # 128-node windows per core
        return (self.NPC + 127) // 128

    @property
    def WSLOTS(self):       # padded node slots per core
        return self.NWIN * 128

    @property
    def PADN(self):         # padded rows of the replicated h table
        return self.CORES * self.WSLOTS

    @property
    def groups(self):
        gs = []
        w = 0
        while w < self.NWIN:
            gs.append(list(range(w, min(w + self.GROUP_W, self.NWIN))))
            w += self.GROUP_W
        return gs


@dataclass
class Prep:
    caps: np.ndarray            # [NWIN, 2] chunks per (window, half)
    chunk_base: np.ndarray      # [NWIN, 2] global chunk index of first chunk
    gathers: list               # per (group, half): (half, chunk_off, nch)
    CHT: int
    idx_all: np.ndarray         # [CORES, 128, 8*CHT] int16
    S_all: np.ndarray           # [CORES, 128, CHT*128] fp8
    deg_inv: np.ndarray         # [CORES, 128, NWIN] fp32
    slot_node: np.ndarray       # [CORES, WSLOTS] node id per slot (-1 pad)


def _balanced_slots(deg: np.ndarray, cfg: Cfg) -> np.ndarray:
    """Assign nodes to (core, window, slot) balancing per-window in-degree
    load. Returns slot_node [CORES, WSLOTS] with -1 padding."""
    import heapq
    nwin_all = cfg.CORES * cfg.NWIN
    loads = [(0, wg) for wg in range(nwin_all)]
    heapq.heapify(loads)
    slots_left = np.full(nwin_all, 128, np.int64)
    core_left = np.full(cfg.CORES, cfg.NPC, np.int64)
    slot_node = np.full((cfg.CORES, cfg.WSLOTS), -1, np.int64)
    order = np.argsort(-deg, kind="stable")
    for n in order:
        held = []
        while True:
            load, wg = heapq.heappop(loads)
            k = wg // cfg.NWIN
            if slots_left[wg] > 0 and core_left[k] > 0:
                w_ = wg % cfg.NWIN
                p = 128 - slots_left[wg]
                slot_node[k, w_ * 128 + p] = n
                slots_left[wg] -= 1
                core_left[k] -= 1
                heapq.heappush(loads, (load + int(deg[n]), wg))
                break
            held.append((load, wg))
            # window/core full: drop it from the heap permanently if full
            if slots_left[wg] > 0 and core_left[k] == 0:
                continue
        for it in held:
            k2 = it[1] // cfg.NWIN
            if slots_left[it[1]] > 0 and core_left[k2] > 0:
                heapq.heappush(loads, it)
    return slot_node


def preprocess(edge_index: np.ndarray, cfg: Cfg) -> Prep:
    src = np.asarray(edge_index[0]).astype(np.int64)
    dst = np.asarray(edge_index[1]).astype(np.int64)
    degc = np.bincount(dst, minlength=cfg.N)
    deg = np.maximum(degc, 1).astype(np.float32)
    assert cfg.SPLIT <= 32768 and cfg.PADN - cfg.SPLIT <= 32768, \
        "both gather tables must be indexable by int16"

    slot_node = _balanced_slots(degc.astype(np.int64), cfg)
    # node -> global slot id
    gslot = np.zeros(cfg.N, np.int64)
    flat = slot_node.reshape(-1)
    valid = flat >= 0
    gslot[flat[valid]] = np.arange(cfg.CORES * cfg.WSLOTS)[valid]

    pad_src = gslot[src]
    dslot = gslot[dst]
    core = dslot // cfg.WSLOTS
    local = dslot - core * cfg.WSLOTS
    w = local >> 7
    j = local & 127
    half = (pad_src >= cfg.SPLIT).astype(np.int64)

    # sort by (core, w, half, src) then dedup identical (src) per (core,w,half)
    order = np.lexsort((j, pad_src, half, w, core))
    s_src = pad_src[order]
    s_j = j[order]
    s_key = ((core * cfg.NWIN + w) * 2 + half)[order]
    # unique (key, src) pairs -> gather items; each item scatters to >=1 dst
    new_item = np.concatenate(
        [[True], (s_key[1:] != s_key[:-1]) | (s_src[1:] != s_src[:-1])])
    item_id = np.cumsum(new_item) - 1          # per edge -> item
    n_items = int(item_id[-1]) + 1
    it_src = s_src[new_item]
    it_key = s_key[new_item]
    nkeys = cfg.CORES * cfg.NWIN * 2
    counts = np.bincount(it_key, minlength=nkeys).reshape(cfg.CORES, cfg.NWIN, 2)
    cum = np.concatenate([[0], np.cumsum(np.bincount(it_key, minlength=nkeys))])

    caps = -(-counts.max(axis=0) // 128)        # [NWIN, 2] ceil over cores
    chunk_base = np.zeros((cfg.NWIN, 2), np.int64)
    gathers = []
    nxt = 0
    for g in cfg.groups:
        for h in (0, 1):
            off = nxt
            for w_ in g:
                chunk_base[w_, h] = nxt
                nxt += caps[w_, h]
            gathers.append((h, off, nxt - off))
    CHT = int(nxt)

    idx_all = np.zeros((cfg.CORES, 128, 8 * CHT), np.int16)
    S_host = np.zeros((cfg.CORES, 128, CHT * 128), np.float16)
    deg_inv = np.ones((cfg.CORES, 128, cfg.NWIN), np.float32)

    g_of_chunk = np.zeros(CHT, np.int64)
    for (h, off, nch) in gathers:
        g_of_chunk[off:off + nch] = off

    # per-item slot within its (k,w,h) run
    item_pos = np.arange(n_items) - cum[it_key]
    # edge -> its item's (chunk, part)
    kk = it_key // (cfg.NWIN * 2)
    ww = (it_key // 2) % cfg.NWIN
    hh = it_key & 1
    it_chunk = chunk_base[ww, hh] + (item_pos >> 7)
    it_part = item_pos & 127
    # scatter S multiplicities (edge-level, duplicates accumulate)
    e_chunk = it_chunk[item_id]
    e_part = it_part[item_id]
    e_core = kk[item_id]
    np.add.at(S_host, (e_core, e_part, e_chunk * 128 + s_j), 1.0)
    # gather idx tables (item-level)
    i_g = (it_chunk - g_of_chunk[it_chunk]) * 128 + it_part
    col = 8 * g_of_chunk[it_chunk] + (i_g >> 4)
    row = i_g & 15
    idx_val = (it_src - np.where(hh == 1, cfg.SPLIT, 0)).astype(np.int16)
    for r in range(8):
        idx_all[kk, 16 * r + row, col] = idx_val

    for k in range(cfg.CORES):
        nodes = slot_node[k]
        real = nodes >= 0
        nd = np.arange(cfg.WSLOTS)[real]
        deg_inv[k, nd & 127, nd >> 7] = 1.0 / deg[nodes[real]]

    return Prep(caps=caps, chunk_base=chunk_base, gathers=gathers, CHT=CHT,
                idx_all=idx_all, S_all=S_host.astype(ml_dtypes.float8_e4m3),
                deg_inv=deg_inv, slot_node=slot_node)


def build_program(cfg: Cfg, prep: Prep, apply_gb: bool):
    nc = bacc.Bacc("TRN2", target_bir_lowering=False, debug=False,
                   num_devices=cfg.CORES, num_swdge_queues=4)
    L, D, CHT, NWIN = cfg.LAYERS, cfg.D, prep.CHT, cfg.NWIN

    # ---- I/O ----
    xT_in = nc.dram_tensor("xT", [128, cfg.WSLOTS], F32, kind="ExternalInput").ap()
    S_in = nc.dram_tensor("S", [128, CHT * 128], F8, kind="ExternalInput").ap()
    idx_in = nc.dram_tensor("idx", [128, 8 * CHT], I16, kind="ExternalInput").ap()
    deg_in = nc.dram_tensor("deg", [128, NWIN], F32, kind="ExternalInput").ap()
    inw_in = nc.dram_tensor("inw", [128, 128], F32, kind="ExternalInput").ap()
    inb_in = nc.dram_tensor("inb", [128, 128], F32, kind="ExternalInput").ap()
    w1_in = nc.dram_tensor("w1", [128, L * 128], F16, kind="ExternalInput").ap()
    w2_in = nc.dram_tensor("w2", [128, L * 128], F16, kind="ExternalInput").ap()
    b1_in = nc.dram_tensor("b1", [128, L], F32, kind="ExternalInput").ap()
    b2_in = nc.dram_tensor("b2", [128, L], F32, kind="ExternalInput").ap()
    id_in = nc.dram_tensor("ident", [128, 128], F16, kind="ExternalInput").ap()
    g_in = nc.dram_tensor("grep", [128, L * 128], F32, kind="ExternalInput").ap()
    bb_in = nc.dram_tensor("brep", [128, L * 128], F32, kind="ExternalInput").ap()
    out_t = nc.dram_tensor("h_out", [cfg.WSLOTS, 128], F32, kind="ExternalOutput").ap()

    # ---- internal DRAM ----
    shards = [nc.dram_tensor(f"shard{l}", [cfg.WSLOTS, 128], F16).ap()
              for l in range(L)]
    hfulls = [nc.dram_tensor(f"hfull{l}", [cfg.PADN, 128], F16,
                             addr_space="Shared").ap()
              for l in range(L)]

    grp_nch = []
    for gi, g in enumerate(cfg.groups):
        nlo = int(prep.caps[g, 0].sum())
        nhi = int(prep.caps[g, 1].sum())
        grp_nch.append((nlo, nhi))
    max_nch = max(a + b for a, b in grp_nch)

    with TileKernel(nc) as tc, ExitStack() as ctx:
        cp = ctx.enter_context(tc.tile_pool(name="const", bufs=1))
        # resident tiles
        S_t = cp.tile([128, CHT * 128], F8)
        idx_t = cp.tile([128, 8 * CHT], I16)
        h_loc = cp.tile([128, NWIN * 128], F32)
        deg_t = cp.tile([128, NWIN], F32)
        inw_t = cp.tile([128, 128], F32)
        inb_t = cp.tile([128, 128], F32)
        w1_t = cp.tile([128, L * 128], F16)
        w2_t = cp.tile([128, L * 128], F16)
        b1_t = cp.tile([128, L], F32)
        b2_t = cp.tile([128, L], F32)
        id_t = cp.tile([128, 128], F16)
        g_t = cp.tile([128, L * 128], F32)
        bb_t = cp.tile([128, L * 128], F32)
        eps_t = cp.tile([128, 1], F32)
        nc.vector.memset(eps_t[:], cfg.LN_EPS)

        nc.sync.dma_start(S_t[:], S_in)
        nc.sync.dma_start(idx_t[:], idx_in)
        nc.sync.dma_start(deg_t[:], deg_in)
        nc.sync.dma_start(inw_t[:], inw_in)
        nc.sync.dma_start(inb_t[:], inb_in)
        nc.sync.dma_start(w1_t[:], w1_in)
        nc.sync.dma_start(w2_t[:], w2_in)
        nc.sync.dma_start(b1_t[:], b1_in)
        nc.sync.dma_start(b2_t[:], b2_in)
        nc.sync.dma_start(id_t[:], id_in)
        if apply_gb:
            nc.sync.dma_start(g_t[:], g_in)
            nc.sync.dma_start(bb_t[:], bb_in)

        # pools
        gp = ctx.enter_context(tc.tile_pool(name="G", bufs=3))
        wp = ctx.enter_context(tc.tile_pool(name="wrk", bufs=3))
        sp = ctx.enter_context(tc.tile_pool(name="small", bufs=4))
        pm = ctx.enter_context(tc.tile_pool(name="pm", bufs=2, space="PSUM"))
        pt = ctx.enter_context(tc.tile_pool(name="pt", bufs=2, space="PSUM"))
        pz = ctx.enter_context(tc.tile_pool(name="pz", bufs=2, space="PSUM"))

        # ---- prologue: h0 = x @ in_w + in_b (node-major windows) ----
        with tc.tile_pool(name="xp", bufs=1) as xp:
            xT_t = xp.tile([128, cfg.WSLOTS], F32)
            nc.sync.dma_start(xT_t[:], xT_in)
            for w_ in range(NWIN):
                h0 = pz.tile([128, 128], F32, tag="mm")
                nc.tensor.matmul(h0[:], xT_t[:, w_ * 128:(w_ + 1) * 128],
                                 inw_t[:], start=True, stop=True)
                nc.vector.tensor_tensor(h_loc[:, w_ * 128:(w_ + 1) * 128],
                                        h0[:], inb_t[:], mybir.AluOpType.add)

        def shard_allgather(l):
            # fp32 SBUF (node window-major) -> fp16 node-major DRAM, then gather
            nc.gpsimd.dma_start(
                shards[l].rearrange("(w p) f -> p w f", p=128),
                h_loc[:].rearrange("p (w f) -> p w f", f=128),
            )
            nc.gpsimd.collective_compute(
                "AllGather", mybir.AluOpType.bypass,
                replica_groups=[list(range(cfg.CORES))],
                ins=[shards[l]], outs=[hfulls[l]],
            )

        shard_allgather(0)

        # ---- layers ----
        for l in range(L):
            hsrc = hfulls[l]
            lo_tbl = hsrc[0:cfg.SPLIT, :]
            hi_tbl = hsrc[cfg.SPLIT:cfg.PADN, :]
            for gi, g in enumerate(cfg.groups):
                nlo, nhi = grp_nch[gi]
                nch = nlo + nhi
                G_t = gp.tile([128, max_nch, D], F16, tag="G")
                goff = int(prep.chunk_base[g[0], 0])  # first chunk of group
                for (h, coff, n) in [(0, goff, nlo), (1, goff + nlo, nhi)]:
                    if n == 0:
                        continue
                    tbl = lo_tbl if h == 0 else hi_tbl
                    # <=8 chunks (1024 idxs = 64 descs/engine) per gather so
                    # single-packet mode stays within the 64-desc packet limit;
                    # rotate the 4 SWDGE queues to parallelize Q7 desc-gen
                    for si, c0 in enumerate(range(coff, coff + n, 8)):
                        nn_ = min(8, coff + n - c0)
                        nc.gpsimd.dma_gather(
                            G_t[:, c0 - goff:c0 - goff + nn_, :], tbl,
                            idx_t[:, 8 * c0:8 * (c0 + nn_)],
                            num_idxs=128 * nn_, num_idxs_reg=128 * nn_,
                            elem_size=D, single_packet=True,
                            queue_num=(gi * 2 + h * 7 + si) % 4,
                        )
                for w_ in g:
                    # scatter: m[dst, f] = sum_c S_c^T @ G_c  (PSUM fp32)
                    chunks = []
                    for h in (0, 1):
                        b0 = int(prep.chunk_base[w_, h])
                        chunks += list(range(b0, b0 + int(prep.caps[w_, h])))
                    m_ps = pm.tile([128, D], F32, tag="m")
                    for ci, c in enumerate(chunks):
                        nc.tensor.matmul(
                            m_ps[:], S_t[:, c * 128:(c + 1) * 128],
                            G_t[:, c - goff, :],
                            start=(ci == 0), stop=(ci == len(chunks) - 1),
                        )
                    # mhat = m * deg_inv  (per-dst scalar), evac -> fp16
                    mhat = wp.tile([128, D], F16, tag="mhat")
                    nc.vector.tensor_scalar(mhat[:], m_ps[:],
                                            deg_t[:, w_:w_ + 1], None,
                                            mybir.AluOpType.mult)
                    # transpose to [feat, nodes]
                    mT_ps = pt.tile([128, D], F16, tag="tr")
                    nc.tensor.transpose(mT_ps[:], mhat[:], id_t[:])
                    mT = wp.tile([128, D], F16, tag="mTs")
                    nc.scalar.copy(mT[:], mT_ps[:])
                    # z1 = relu(W1^T mT + b1)
                    z1_ps = pz.tile([128, D], F32, tag="mm")
                    nc.tensor.matmul(z1_ps[:], w1_t[:, l * 128:(l + 1) * 128],
                                     mT[:], start=True, stop=True)
                    z1 = wp.tile([128, D], F16, tag="z1s")
                    nc.scalar.activation(z1[:], z1_ps[:],
                                         mybir.ActivationFunctionType.Relu,
                                         bias=b1_t[:, l:l + 1])
                    # z2 = W2^T z1 + b2
                    z2_ps = pz.tile([128, D], F32, tag="mm")
                    nc.tensor.matmul(z2_ps[:], w2_t[:, l * 128:(l + 1) * 128],
                                     z1[:], start=True, stop=True)
                    z2t = wp.tile([128, D], F16, tag="z2t")
                    nc.scalar.activation(z2t[:], z2_ps[:],
                                         mybir.ActivationFunctionType.Identity,
                                         bias=b2_t[:, l:l + 1])
                    # back to [nodes, feat]
                    z2n_ps = pt.tile([128, D], F16, tag="tr")
                    nc.tensor.transpose(z2n_ps[:], z2t[:], id_t[:])
                    # residual + stats
                    hw = h_loc[:, w_ * 128:(w_ + 1) * 128]
                    r = wp.tile([128, D], F32, tag="r")
                    rs = sp.tile([128, 1], F32, tag="rs")
                    nc.vector.scalar_tensor_tensor(
                        r[:], hw, 0.0, z2n_ps[:],
                        mybir.AluOpType.bypass, mybir.AluOpType.add,
                        accum_out=rs[:])
                    rsq = wp.tile([128, D], F32, tag="rsq")
                    rqs = sp.tile([128, 1], F32, tag="rqs")
                    nc.scalar.activation(rsq[:], r[:],
                                         mybir.ActivationFunctionType.Square,
                                         accum_out=rqs[:])
                    mu = sp.tile([128, 1], F32, tag="mu")
                    nc.vector.tensor_scalar(mu[:], rs[:], 1.0 / D, None,
                                            mybir.AluOpType.mult)
                    mu2 = sp.tile([128, 1], F32, tag="mu2")
                    nc.vector.tensor_tensor(mu2[:], mu[:], mu[:],
                                            mybir.AluOpType.mult)
                    var = sp.tile([128, 1], F32, tag="var")
                    nc.vector.scalar_tensor_tensor(
                        var[:], rqs[:], 1.0 / D, mu2[:],
                        mybir.AluOpType.mult, mybir.AluOpType.subtract)
                    sd = sp.tile([128, 1], F32, tag="sd")
                    nc.scalar.activation(sd[:], var[:],
                                         mybir.ActivationFunctionType.Sqrt,
                                         bias=eps_t[:])
                    inv = sp.tile([128, 1], F32, tag="inv")
                    nc.vector.reciprocal(inv[:], sd[:])
                    if apply_gb:
                        t1 = wp.tile([128, D], F32, tag="t1")
                        nc.vector.tensor_scalar(t1[:], r[:], mu[:], inv[:],
                                                mybir.AluOpType.subtract,
                                                mybir.AluOpType.mult)
                        t2 = wp.tile([128, D], F32, tag="t2")
                        nc.vector.tensor_tensor(
                            t2[:], t1[:], g_t[:, l * 128:(l + 1) * 128],
                            mybir.AluOpType.mult)
                        nc.vector.tensor_tensor(
                            hw, t2[:], bb_t[:, l * 128:(l + 1) * 128],
                            mybir.AluOpType.add)
                    else:
                        nc.vector.tensor_scalar(hw, r[:], mu[:], inv[:],
                                                mybir.AluOpType.subtract,
                                                mybir.AluOpType.mult)
            if l + 1 < L:
                shard_allgather(l + 1)

        # ---- epilogue: write fp32 shard ----
        nc.sync.dma_start(
            out_t.rearrange("(w p) f -> p w f", p=128),
            h_loc[:].rearrange("p (w f) -> p w f", f=128),
        )

    nc.compile()
    return nc


def TileKernel(nc):
    return tile.TileContext(nc)


_CACHE = {}


def _prepare(inputs, cfg: Cfg):
    edge_index = np.asarray(inputs["edge_index"])
    key = hashlib.sha1(edge_index.tobytes()).hexdigest()
    ln_g = np.asarray(inputs["ln_g"], np.float32)
    ln_b = np.asarray(inputs["ln_b"], np.float32)
    apply_gb = not (np.all(ln_g == 1.0) and np.all(ln_b == 0.0))
    key = (key, apply_gb, cfg.N, cfg.CORES)
    if key not in _CACHE:
        prep = preprocess(edge_index, cfg)
        nc = build_program(cfg, prep, apply_gb)
        _CACHE[key] = (prep, nc, apply_gb)
    return _CACHE[key]


def _in_maps(inputs, cfg: Cfg, prep: Prep, apply_gb: bool):
    x = np.asarray(inputs["x"], np.float32)
    L = cfg.LAYERS
    inw = np.asarray(inputs["in_w"], np.float32)
    inb = np.asarray(inputs["in_b"], np.float32)
    w1 = np.asarray(inputs["w1"], np.float32)
    w2 = np.asarray(inputs["w2"], np.float32)
    b1 = np.asarray(inputs["b1"], np.float32)
    b2 = np.asarray(inputs["b2"], np.float32)
    ln_g = np.asarray(inputs["ln_g"], np.float32)
    ln_b = np.asarray(inputs["ln_b"], np.float32)

    inb_rep = np.tile(inb[None, :], (128, 1))
    w1_pack = np.concatenate([w1[l] for l in range(L)], axis=1).astype(np.float16)
    w2_pack = np.concatenate([w2[l] for l in range(L)], axis=1).astype(np.float16)
    b1_cols = np.ascontiguousarray(b1.T)           # [128, L]
    b2_cols = np.ascontiguousarray(b2.T)
    g_rep = np.concatenate([np.tile(ln_g[l][None, :], (128, 1)) for l in range(L)], axis=1)
    b_rep = np.concatenate([np.tile(ln_b[l][None, :], (128, 1)) for l in range(L)], axis=1)
    ident = np.eye(128, dtype=np.float16)

    maps = []
    for k in range(cfg.CORES):
        xT = np.zeros((128, cfg.WSLOTS), np.float32)
        nodes = prep.slot_node[k]
        vmask = nodes >= 0
        xT[:, vmask] = x[nodes[vmask]].T
        maps.append({
            "xT": xT,
            "S": np.ascontiguousarray(prep.S_all[k]),
            "idx": np.ascontiguousarray(prep.idx_all[k]),
            "deg": np.ascontiguousarray(prep.deg_inv[k]),
            "inw": inw, "inb": inb_rep,
            "w1": w1_pack, "w2": w2_pack,
            "b1": b1_cols, "b2": b2_cols,
            "ident": ident, "grep": g_rep, "brep": b_rep,
        })
    return maps


def _run(inputs, cfg=None, trace=False):
    cfg = cfg or Cfg()
    prep, nc, apply_gb = _prepare(inputs, cfg)
    maps = _in_maps(inputs, cfg, prep, apply_gb)
    res = bass_utils.run_bass_kernel_spmd(
        nc, maps, core_ids=list(range(cfg.CORES)),
        trace=trace, trace_cores=list(range(cfg.CORES)) if trace else None,
    )
    out = np.empty((cfg.N, cfg.D), np.float32)
    for k in range(cfg.CORES):
        nodes = prep.slot_node[k]
        vmask = nodes >= 0
        out[nodes[vmask]] = res.results[k]["h_out"][vmask]
    return out, res


def kernel(**inputs) -> np.ndarray:
    out, _ = _run(inputs)
    return out


def kernel_profiled(**inputs):
    out, res = _run(inputs, trace=True)
    return out, res


# revision 12
# speedup vs baseline: 1.9545x; 1.0499x over previous
"""LiteGearNet GNN message-passing kernel for 8 Trainium2 NeuronCores.

Strategy (matches the sharding hint: partition nodes, replicate weights):
 - Nodes are partitioned across the 8 cores (6250 each). Each core computes
   its nodes' messages, MLP and LayerNorm. The small 128x128 weights are
   replicated.
 - The gather h[src] is a hardware `dma_gather` (SWDGE) from a per-core
   HBM replica of h (fp16). Edges are pre-sorted by destination window
   (128 dst nodes) on the host, so the scatter-add becomes a sequence of
   one-hot matmuls accumulating in PSUM (S^T @ G per 128-edge chunk),
   with the one-hot S matrices precomputed on the host in fp8 and kept
   resident in SBUF.
 - After each layer, cores AllGather their fp16 shard of the new h into a
   shared HBM buffer that serves as the next layer's gather source.
 - Precision: gather/matmul operands fp16/fp8(one-hot exact), PSUM/MLP
   accumulation + LayerNorm in fp32, residual master copy of h in fp32.
"""
import hashlib
from contextlib import ExitStack
from dataclasses import dataclass, field

import numpy as np
import ml_dtypes

import concourse.bass as bass
import concourse.tile as tile
from concourse import bacc, mybir
from concourse import bass_utils

F32 = mybir.dt.float32
F16 = mybir.dt.float16
F8 = mybir.dt.float8e4
I16 = mybir.dt.int16


@dataclass
class Cfg:
    N: int = 50000          # nodes
    D: int = 128            # feature dim
    LAYERS: int = 3
    CORES: int = 8
    LN_EPS: float = 1e-5
    SPLIT: int = 32768      # int16 gather-table split
    GROUP_W: int = 3        # windows per gather group

    @property
    def NPC(self):          # nodes per core
        return self.N // self.CORES

    @property
    def NWIN(self):         # 128-node windows per core
        return (self.NPC + 127) // 128

    @property
    def WSLOTS(self):       # padded node slots per core
        return self.NWIN * 128

    @property
    def PADN(self):         # padded rows of the replicated h table
        return self.CORES * self.WSLOTS

    @property
    def groups(self):
        gs = []
        w = 0
        while w < self.NWIN:
            gs.append(list(range(w, min(w + self.GROUP_W, self.NWIN))))
            w += self.GROUP_W
        return gs


@dataclass
class Prep:
    caps: np.ndarray            # [NWIN, 2] chunks per (window, half)
    chunk_base: np.ndarray      # [NWIN, 2] global chunk index of first chunk
    gathers: list               # per (group, half): (half, chunk_off, nch)
    CHT: int
    idx_all: np.ndarray         # [CORES, 128, 8*CHT] int16
    S_all: np.ndarray           # [CORES, 128, CHT*128] fp8
    deg_inv: np.ndarray         # [CORES, 128, NWIN] fp32
    slot_node: np.ndarray       # [CORES, WSLOTS] node id per slot (-1 pad)


def _balanced_slots(deg: np.ndarray, cfg: Cfg) -> np.ndarray:
    """Assign nodes to (core, window, slot) balancing per-window in-degree
    load. Returns slot_node [CORES, WSLOTS] with -1 padding."""
    import heapq
    nwin_all = cfg.CORES * cfg.NWIN
    loads = [(0, wg) for wg in range(nwin_all)]
    heapq.heapify(loads)
    slots_left = np.full(nwin_all, 128, np.int64)
    core_left = np.full(cfg.CORES, cfg.NPC, np.int64)
    slot_node = np.full((cfg.CORES, cfg.WSLOTS), -1, np.int64)
    order = np.argsort(-deg, kind="stable")
    for n in order:
        held = []
        while True:
            load, wg = heapq.heappop(loads)
            k = wg // cfg.NWIN
            if slots_left[wg] > 0 and core_left[k] > 0:
                w_ = wg % cfg.NWIN
                p = 128 - slots_left[wg]
                slot_node[k, w_ * 128 + p] = n
                slots_left[wg] -= 1
                core_left[k] -= 1
                heapq.heappush(loads, (load + int(deg[n]), wg))
                break
            held.append((load, wg))
            # window/core full: drop it from the heap permanently if full
            if slots_left[wg] > 0 and core_left[k] == 0:
                continue
        for it in held:
            k2 = it[1] // cfg.NWIN
            if slots_left[it[1]] > 0 and core_left[k2] > 0:
                heapq.heappush(loads, it)
    return slot_node


def preprocess(edge_index: np.ndarray, cfg: Cfg) -> Prep:
    src = np.asarray(edge_index[0]).astype(np.int64)
    dst = np.asarray(edge_index[1]).astype(np.int64)
    degc = np.bincount(dst, minlength=cfg.N)
    deg = np.maximum(degc, 1).astype(np.float32)
    assert cfg.SPLIT <= 32768 and cfg.PADN - cfg.SPLIT <= 32768, \
        "both gather tables must be indexable by int16"

    slot_node = _balanced_slots(degc.astype(np.int64), cfg)
    # node -> global slot id
    gslot = np.zeros(cfg.N, np.int64)
    flat = slot_node.reshape(-1)
    valid = flat >= 0
    gslot[flat[valid]] = np.arange(cfg.CORES * cfg.WSLOTS)[valid]

    pad_src = gslot[src]
    dslot = gslot[dst]
    core = dslot // cfg.WSLOTS
    local = dslot - core * cfg.WSLOTS
    w = local >> 7
    j = local & 127
    half = (pad_src >= cfg.SPLIT).astype(np.int64)

    # sort by (core, w, half, src) then dedup identical (src) per (core,w,half)
    order = np.lexsort((j, pad_src, half, w, core))
    s_src = pad_src[order]
    s_j = j[order]
    s_key = ((core * cfg.NWIN + w) * 2 + half)[order]
    # unique (key, src) pairs -> gather items; each item scatters to >=1 dst
    new_item = np.concatenate(
        [[True], (s_key[1:] != s_key[:-1]) | (s_src[1:] != s_src[:-1])])
    item_id = np.cumsum(new_item) - 1          # per edge -> item
    n_items = int(item_id[-1]) + 1
    it_src = s_src[new_item]
    it_key = s_key[new_item]
    nkeys = cfg.CORES * cfg.NWIN * 2
    counts = np.bincount(it_key, minlength=nkeys).reshape(cfg.CORES, cfg.NWIN, 2)
    cum = np.concatenate([[0], np.cumsum(np.bincount(it_key, minlength=nkeys))])

    caps = -(-counts.max(axis=0) // 128)        # [NWIN, 2] ceil over cores
    chunk_base = np.zeros((cfg.NWIN, 2), np.int64)
    gathers = []
    nxt = 0
    for g in cfg.groups:
        for h in (0, 1):
            off = nxt
            for w_ in g:
                chunk_base[w_, h] = nxt
                nxt += caps[w_, h]
            gathers.append((h, off, nxt - off))
    CHT = int(nxt)

    idx_all = np.zeros((cfg.CORES, 128, 8 * CHT), np.int16)
    S_host = np.zeros((cfg.CORES, 128, CHT * 128), np.float16)
    deg_inv = np.ones((cfg.CORES, 128, cfg.NWIN), np.float32)

    g_of_chunk = np.zeros(CHT, np.int64)
    for (h, off, nch) in gathers:
        g_of_chunk[off:off + nch] = off

    # per-item slot within its (k,w,h) run
    item_pos = np.arange(n_items) - cum[it_key]
    # edge -> its item's (chunk, part)
    kk = it_key // (cfg.NWIN * 2)
    ww = (it_key // 2) % cfg.NWIN
    hh = it_key & 1
    it_chunk = chunk_base[ww, hh] + (item_pos >> 7)
    it_part = item_pos & 127
    # scatter S multiplicities (edge-level, duplicates accumulate)
    e_chunk = it_chunk[item_id]
    e_part = it_part[item_id]
    e_core = kk[item_id]
    np.add.at(S_host, (e_core, e_part, e_chunk * 128 + s_j), 1.0)
    # gather idx tables (item-level)
    i_g = (it_chunk - g_of_chunk[it_chunk]) * 128 + it_part
    col = 8 * g_of_chunk[it_chunk] + (i_g >> 4)
    row = i_g & 15
    idx_val = (it_src - np.where(hh == 1, cfg.SPLIT, 0)).astype(np.int16)
    for r in range(8):
        idx_all[kk, 16 * r + row, col] = idx_val

    for k in range(cfg.CORES):
        nodes = slot_node[k]
        real = nodes >= 0
        nd = np.arange(cfg.WSLOTS)[real]
        deg_inv[k, nd & 127, nd >> 7] = 1.0 / deg[nodes[real]]

    return Prep(caps=caps, chunk_base=chunk_base, gathers=gathers, CHT=CHT,
                idx_all=idx_all, S_all=S_host.astype(ml_dtypes.float8_e4m3),
                deg_inv=deg_inv, slot_node=slot_node)


def build_program(cfg: Cfg, prep: Prep, apply_gb: bool):
    nc = bacc.Bacc("TRN2", target_bir_lowering=False, debug=False,
                   num_devices=cfg.CORES, num_swdge_queues=4)
    L, D, CHT, NWIN = cfg.LAYERS, cfg.D, prep.CHT, cfg.NWIN

    # ---- I/O ----
    xT_in = nc.dram_tensor("xT", [128, cfg.WSLOTS], F32, kind="ExternalInput").ap()
    S_in = nc.dram_tensor("S", [128, CHT * 128], F8, kind="ExternalInput").ap()
    idx_in = nc.dram_tensor("idx", [128, 8 * CHT], I16, kind="ExternalInput").ap()
    deg_in = nc.dram_tensor("deg", [128, NWIN], F32, kind="ExternalInput").ap()
    inw_in = nc.dram_tensor("inw", [128, 128], F32, kind="ExternalInput").ap()
    inb_in = nc.dram_tensor("inb", [128, 128], F32, kind="ExternalInput").ap()
    w1_in = nc.dram_tensor("w1", [128, L * 128], F16, kind="ExternalInput").ap()
    w2_in = nc.dram_tensor("w2", [128, L * 128], F16, kind="ExternalInput").ap()
    b1_in = nc.dram_tensor("b1", [128, L], F32, kind="ExternalInput").ap()
    b2_in = nc.dram_tensor("b2", [128, L], F32, kind="ExternalInput").ap()
    id_in = nc.dram_tensor("ident", [128, 128], F16, kind="ExternalInput").ap()
    g_in = nc.dram_tensor("grep", [128, L * 128], F32, kind="ExternalInput").ap()
    bb_in = nc.dram_tensor("brep", [128, L * 128], F32, kind="ExternalInput").ap()
    out_t = nc.dram_tensor("h_out", [cfg.WSLOTS, 128], F16, kind="ExternalOutput").ap()

    # ---- internal DRAM ----
    shards = [nc.dram_tensor(f"shard{l}", [cfg.WSLOTS, 128], F16).ap()
              for l in range(L)]
    hfulls = [nc.dram_tensor(f"hfull{l}", [cfg.PADN, 128], F16,
                             addr_space="Shared").ap()
              for l in range(L)]

    grp_nch = []
    for gi, g in enumerate(cfg.groups):
        nlo = int(prep.caps[g, 0].sum())
        nhi = int(prep.caps[g, 1].sum())
        grp_nch.append((nlo, nhi))
    max_nch = max(a + b for a, b in grp_nch)

    with TileKernel(nc) as tc, ExitStack() as ctx:
        cp = ctx.enter_context(tc.tile_pool(name="const", bufs=1))
        # resident tiles
        S_t = cp.tile([128, CHT * 128], F8)
        idx_t = cp.tile([128, 8 * CHT], I16)
        h_loc = cp.tile([128, NWIN * 128], F16)
        h0sum = cp.tile([128, NWIN], F32)
        ones_c = cp.tile([128, 1], F16)
        nc.vector.memset(ones_c[:], 1.0)
        deg_t = cp.tile([128, NWIN], F32)
        inw_t = cp.tile([128, 128], F32)
        inb_t = cp.tile([128, 128], F32)
        w1_t = cp.tile([128, L * 128], F16)
        w2_t = cp.tile([128, L * 128], F16)
        b1_t = cp.tile([128, L], F32)
        b2_t = cp.tile([128, L], F32)
        id_t = cp.tile([128, 128], F16)
        g_t = cp.tile([128, L * 128], F32)
        bb_t = cp.tile([128, L * 128], F32)
        eps_t = cp.tile([128, 1], F32)
        nc.vector.memset(eps_t[:], cfg.LN_EPS)

        nc.sync.dma_start(S_t[:], S_in)
        nc.sync.dma_start(idx_t[:], idx_in)
        nc.sync.dma_start(deg_t[:], deg_in)
        nc.sync.dma_start(inw_t[:], inw_in)
        nc.sync.dma_start(inb_t[:], inb_in)
        nc.sync.dma_start(w1_t[:], w1_in)
        nc.sync.dma_start(w2_t[:], w2_in)
        nc.sync.dma_start(b1_t[:], b1_in)
        nc.sync.dma_start(b2_t[:], b2_in)
        nc.sync.dma_start(id_t[:], id_in)
        if apply_gb:
            nc.sync.dma_start(g_t[:], g_in)
            nc.sync.dma_start(bb_t[:], bb_in)

        # pools
        gp = ctx.enter_context(tc.tile_pool(name="G", bufs=3))
        wp = ctx.enter_context(tc.tile_pool(name="wrk", bufs=3))
        sp = ctx.enter_context(tc.tile_pool(name="small", bufs=4))
        pm = ctx.enter_context(tc.tile_pool(name="pm", bufs=2, space="PSUM"))
        pt = ctx.enter_context(tc.tile_pool(name="pt", bufs=2, space="PSUM"))
        pz = ctx.enter_context(tc.tile_pool(name="pz", bufs=2, space="PSUM"))

        # ---- prologue: h0 = x @ in_w + in_b (node-major windows) ----
        with tc.tile_pool(name="xp", bufs=1) as xp:
            xT_t = xp.tile([128, cfg.WSLOTS], F32)
            nc.sync.dma_start(xT_t[:], xT_in)
            for w_ in range(NWIN):
                h0 = pz.tile([128, 128], F32, tag="mm")
                nc.tensor.matmul(h0[:], xT_t[:, w_ * 128:(w_ + 1) * 128],
                                 inw_t[:], start=True, stop=True)
                nc.vector.scalar_tensor_tensor(
                    h_loc[:, w_ * 128:(w_ + 1) * 128], h0[:], 0.0, inb_t[:],
                    mybir.AluOpType.bypass, mybir.AluOpType.add,
                    accum_out=h0sum[:, w_:w_ + 1])

        def shard_allgather(l):
            nc.sync.dma_start(
                shards[l].rearrange("(w p) f -> p w f", p=128),
                h_loc[:].rearrange("p (w f) -> p w f", f=128),
            )
            nc.gpsimd.collective_compute(
                "AllGather", mybir.AluOpType.bypass,
                replica_groups=[list(range(cfg.CORES))],
                ins=[shards[l]], outs=[hfulls[l]],
            )

        shard_allgather(0)

        # ---- layers ----
        pool_dma_ctr = [0]
        for l in range(L):
            hsrc = hfulls[l]
            lo_tbl = hsrc[0:cfg.SPLIT, :]
            hi_tbl = hsrc[cfg.SPLIT:cfg.PADN, :]
            for gi, g in enumerate(cfg.groups):
                nlo, nhi = grp_nch[gi]
                nch = nlo + nhi
                G_t = gp.tile([128, max_nch, D], F16, tag="G")
                goff = int(prep.chunk_base[g[0], 0])  # first chunk of group
                for (h, coff, n) in [(0, goff, nlo), (1, goff + nlo, nhi)]:
                    if n == 0:
                        continue
                    tbl = lo_tbl if h == 0 else hi_tbl
                    # <=8 chunks (1024 idxs = 64 descs/engine) per gather so
                    # single-packet mode stays within the 64-desc packet limit.
                    # queue_num must match Tile's DMASW lane round-robin
                    # (lane = pool-DMA trace index % 8, queue = lane % 4).
                    for si, c0 in enumerate(range(coff, coff + n, 8)):
                        nn_ = min(8, coff + n - c0)
                        nc.gpsimd.dma_gather(
                            G_t[:, c0 - goff:c0 - goff + nn_, :], tbl,
                            idx_t[:, 8 * c0:8 * (c0 + nn_)],
                            num_idxs=128 * nn_, num_idxs_reg=128 * nn_,
                            elem_size=D, single_packet=True,
                            queue_num=pool_dma_ctr[0] % 4,
                        )
                        pool_dma_ctr[0] += 1
                for w_ in g:
                    # scatter: m[dst, f] = sum_c S_c^T @ G_c  (PSUM fp32)
                    chunks = []
                    for h in (0, 1):
                        b0 = int(prep.chunk_base[w_, h])
                        chunks += list(range(b0, b0 + int(prep.caps[w_, h])))
                    m_ps = pm.tile([128, D], F32, tag="m")
                    for ci, c in enumerate(chunks):
                        nc.tensor.matmul(
                            m_ps[:], S_t[:, c * 128:(c + 1) * 128],
                            G_t[:, c - goff, :],
                            start=(ci == 0), stop=(ci == len(chunks) - 1),
                        )
                    # mhat = m * deg_inv  (per-dst scalar), evac -> fp16
                    mhat = wp.tile([128, D], F16, tag="mhat")
                    nc.scalar.activation(mhat[:], m_ps[:],
                                         mybir.ActivationFunctionType.Copy,
                                         scale=deg_t[:, w_:w_ + 1])
                    # transpose to [feat, nodes]
                    mT_ps = pt.tile([128, D], F16, tag="tr")
                    nc.tensor.transpose(mT_ps[:], mhat[:], id_t[:])
                    mT = wp.tile([128, D], F16, tag="mTs")
                    nc.scalar.copy(mT[:], mT_ps[:])
                    # z1 = relu(W1^T mT + b1)
                    z1_ps = pz.tile([128, D], F32, tag="mm")
                    nc.tensor.matmul(z1_ps[:], w1_t[:, l * 128:(l + 1) * 128],
                                     mT[:], start=True, stop=True)
                    z1 = wp.tile([128, D], F16, tag="z1s")
                    nc.scalar.activation(z1[:], z1_ps[:],
                                         mybir.ActivationFunctionType.Relu,
                                         bias=b1_t[:, l:l + 1])
                    # z2 = W2^T z1 + b2
                    z2_ps = pz.tile([128, D], F32, tag="mm")
                    nc.tensor.matmul(z2_ps[:], w2_t[:, l * 128:(l + 1) * 128],
                                     z1[:], start=True, stop=True)
                    z2t = wp.tile([128, D], F16, tag="z2t")
                    nc.scalar.activation(z2t[:], z2_ps[:],
                                         mybir.ActivationFunctionType.Identity,
                                         bias=b2_t[:, l:l + 1])
                    # per-node sum(z2) on PE: z2t.T @ ones -> [nodes, 1]
                    mus_ps = pm.tile([128, 1], F32, tag="mus")
                    nc.tensor.matmul(mus_ps[:], z2t[:], ones_c[:],
                                     start=True, stop=True)
                    # back to [nodes, feat]
                    z2n_ps = pt.tile([128, D], F16, tag="tr")
                    nc.tensor.transpose(z2n_ps[:], z2t[:], id_t[:])
                    # residual (no accumulator use on DVE)
                    hw = h_loc[:, w_ * 128:(w_ + 1) * 128]
                    r = wp.tile([128, D], F16, tag="r")
                    mu = sp.tile([128, 1], F32, tag="mu")
                    if apply_gb:
                        # general path: mean(h) != 0, take sums on DVE
                        rs = sp.tile([128, 1], F32, tag="rs")
                        nc.vector.scalar_tensor_tensor(
                            r[:], hw, 0.0, z2n_ps[:],
                            mybir.AluOpType.bypass, mybir.AluOpType.add,
                            accum_out=rs[:])
                        nc.vector.tensor_scalar(mu[:], rs[:], 1.0 / D, None,
                                                mybir.AluOpType.mult)
                    else:
                        nc.vector.tensor_tensor(r[:], hw, z2n_ps[:],
                                                mybir.AluOpType.add)
                    rsq = wp.tile([128, D], F32, tag="rsq")
                    rqs = sp.tile([128, 1], F32, tag="rqs")
                    nc.scalar.activation(rsq[:], r[:],
                                         mybir.ActivationFunctionType.Square,
                                         accum_out=rqs[:])
                    if not apply_gb:
                        # mean(h)=0 exactly when ln_g==1/ln_b==0 (layers>=1);
                        # layer 0 adds the prologue h0 sums
                        if l == 0:
                            ms2 = sp.tile([128, 1], F32, tag="ms2")
                            nc.vector.tensor_tensor(ms2[:], mus_ps[:],
                                                    h0sum[:, w_:w_ + 1],
                                                    mybir.AluOpType.add)
                            nc.vector.tensor_scalar(mu[:], ms2[:], 1.0 / D,
                                                    None, mybir.AluOpType.mult)
                        else:
                            nc.vector.tensor_scalar(mu[:], mus_ps[:], 1.0 / D,
                                                    None, mybir.AluOpType.mult)
                    mu2 = sp.tile([128, 1], F32, tag="mu2")
                    nc.vector.tensor_tensor(mu2[:], mu[:], mu[:],
                                            mybir.AluOpType.mult)
                    var = sp.tile([128, 1], F32, tag="var")
                    nc.vector.scalar_tensor_tensor(
                        var[:], rqs[:], 1.0 / D, mu2[:],
                        mybir.AluOpType.mult, mybir.AluOpType.subtract)
                    sd = sp.tile([128, 1], F32, tag="sd")
                    nc.scalar.activation(sd[:], var[:],
                                         mybir.ActivationFunctionType.Sqrt,
                                         bias=eps_t[:])
                    inv = sp.tile([128, 1], F32, tag="inv")
                    nc.vector.reciprocal(inv[:], sd[:])
                    if apply_gb:
                        t1 = wp.tile([128, D], F32, tag="t1")
                        nc.vector.tensor_scalar(t1[:], r[:], mu[:], inv[:],
                                                mybir.AluOpType.subtract,
                                                mybir.AluOpType.mult)
                        t2 = wp.tile([128, D], F32, tag="t2")
                        nc.vector.tensor_tensor(
                            t2[:], t1[:], g_t[:, l * 128:(l + 1) * 128],
                            mybir.AluOpType.mult)
                        nc.vector.tensor_tensor(
                            hw, t2[:], bb_t[:, l * 128:(l + 1) * 128],
                            mybir.AluOpType.add)
                    else:
                        nc.vector.tensor_scalar(hw, r[:], mu[:], inv[:],
                                                mybir.AluOpType.subtract,
                                                mybir.AluOpType.mult)
            if l + 1 < L:
                shard_allgather(l + 1)

        # ---- epilogue: write fp32 shard ----
        nc.sync.dma_start(
            out_t.rearrange("(w p) f -> p w f", p=128),
            h_loc[:].rearrange("p (w f) -> p w f", f=128),
        )

    nc.compile()
    return nc


def TileKernel(nc):
    return tile.TileContext(nc)


_CACHE = {}


def _prepare(inputs, cfg: Cfg):
    edge_index = np.asarray(inputs["edge_index"])
    key = hashlib.sha1(edge_index.tobytes()).hexdigest()
    ln_g = np.asarray(inputs["ln_g"], np.float32)
    ln_b = np.asarray(inputs["ln_b"], np.float32)
    apply_gb = not (np.all(ln_g == 1.0) and np.all(ln_b == 0.0))
    key = (key, apply_gb, cfg.N, cfg.CORES)
    if key not in _CACHE:
        prep = preprocess(edge_index, cfg)
        nc = build_program(cfg, prep, apply_gb)
        _CACHE[key] = (prep, nc, apply_gb)
    return _CACHE[key]


def _in_maps(inputs, cfg: Cfg, prep: Prep, apply_gb: bool):
    x = np.asarray(inputs["x"], np.float32)
    L = cfg.LAYERS
    inw = np.asarray(inputs["in_w"], np.float32)
    inb = np.asarray(inputs["in_b"], np.float32)
    w1 = np.asarray(inputs["w1"], np.float32)
    w2 = np.asarray(inputs["w2"], np.float32)
    b1 = np.asarray(inputs["b1"], np.float32)
    b2 = np.asarray(inputs["b2"], np.float32)
    ln_g = np.asarray(inputs["ln_g"], np.float32)
    ln_b = np.asarray(inputs["ln_b"], np.float32)

    inb_rep = np.tile(inb[None, :], (128, 1))
    w1_pack = np.concatenate([w1[l] for l in range(L)], axis=1).astype(np.float16)
    w2_pack = np.concatenate([w2[l] for l in range(L)], axis=1).astype(np.float16)
    b1_cols = np.ascontiguousarray(b1.T)           # [128, L]
    b2_cols = np.ascontiguousarray(b2.T)
    g_rep = np.concatenate([np.tile(ln_g[l][None, :], (128, 1)) for l in range(L)], axis=1)
    b_rep = np.concatenate([np.tile(ln_b[l][None, :], (128, 1)) for l in range(L)], axis=1)
    ident = np.eye(128, dtype=np.float16)

    maps = []
    for k in range(cfg.CORES):
        xT = np.zeros((128, cfg.WSLOTS), np.float32)
        nodes = prep.slot_node[k]
        vmask = nodes >= 0
        xT[:, vmask] = x[nodes[vmask]].T
        maps.append({
            "xT": xT,
            "S": np.ascontiguousarray(prep.S_all[k]),
            "idx": np.ascontiguousarray(prep.idx_all[k]),
            "deg": np.ascontiguousarray(prep.deg_inv[k]),
            "inw": inw, "inb": inb_rep,
            "w1": w1_pack, "w2": w2_pack,
            "b1": b1_cols, "b2": b2_cols,
            "ident": ident, "grep": g_rep, "brep": b_rep,
        })
    return maps


def _run(inputs, cfg=None, trace=False):
    cfg = cfg or Cfg()
    prep, nc, apply_gb = _prepare(inputs, cfg)
    maps = _in_maps(inputs, cfg, prep, apply_gb)
    res = bass_utils.run_bass_kernel_spmd(
        nc, maps, core_ids=list(range(cfg.CORES)),
        trace=trace, trace_cores=list(range(cfg.CORES)) if trace else None,
    )
    out = np.empty((cfg.N, cfg.D), np.float32)
    for k in range(cfg.CORES):
        nodes = prep.slot_node[k]
        vmask = nodes >= 0
        out[nodes[vmask]] = res.results[k]["h_out"][vmask].astype(np.float32)
    return out, res


def kernel(**inputs) -> np.ndarray:
    out, _ = _run(inputs)
    return out


def kernel_profiled(**inputs):
    out, res = _run(inputs, trace=True)
    return out, res
